# revision 7
# baseline (speedup 1.0000x reference)
"""Trainium2 Bass kernel for the CMoALoRA2B selector (MoE routing).

Math (per the reference):
    x      = input_x.mean(axis=1)                    # [bz, dim]   <- 1 GiB read, the memory-bound part
    l1     = x @ W_b.T                               # [bz, E]
    hidden = embed[loraA_indices].mean(axis=1)       # [bz, dim]
    l2     = hidden @ W_c.T                          # [bz, E]
    logits = softmax(l1) + softmax(l2)
    topk   = top_k(logits, 4)
    out    = softmax(logits) masked to topk, renormalized
           = softmax over the top-4 logits, scattered back     (identical math)

Distribution: data-parallel over batch; 8 cores x 8 batches each. Router
weights (16 x 2048 each) are replicated. Everything runs on-device; the host
only shards inputs and concatenates the per-core [8,16]/[8,4] outputs.

Device strategy per core:
  Stage 1: stream the [8, 2048, 2048] shard in [128, 2048] tiles; each
    128x128 chunk becomes the matmul *stationary* operand against a ones
    [128,1] moving vector, accumulating column sums into a single PSUM tile
    [128, 128] laid out as col = chunk*8 + batch. This yields the transposed
    batch-means directly (dim on partitions), which the router matmuls need.
  Stage 2 (tiny): PE-transpose the 16x2048 weights into [128,16] chunks,
    G = embed @ W_c.T via PE, expert counts from indices via iota+is_equal,
    l2 = counts/4 @ G, softmaxes on ACT (exp + accum_out), top-k via the DVE
    max/max_index instructions, and scatter of the renormalized top-4 scores.
"""

import sys

for _p in ("/opt/trn_rl_repo",):
    if _p not in sys.path:
        sys.path.insert(0, _p)

import numpy as np

BZ, SEQ, DIM = 64, 2048, 2048
E, R = 16, 4
N_CORES = 8
B = BZ // N_CORES  # batches per core
P = 128  # partitions

# Tunables
SEQ_TILES_PER_DMA = 2  # T: seq tiles (128 rows) per dma_start
X_BUFS = 6  # in-flight input tiles


def build_nc(seq=SEQ, t_per_dma=SEQ_TILES_PER_DMA, x_bufs=X_BUFS):
    import concourse.bass as bass
    import concourse.tile as tile
    from concourse import bacc, mybir

    f32 = mybir.dt.float32
    i32 = mybir.dt.int32
    u32 = mybir.dt.uint32
    AF = mybir.ActivationFunctionType
    OP = mybir.AluOpType

    NT = seq // P  # seq tiles per batch
    T = t_per_dma
    NG = NT // T  # dma groups per batch
    NCH = DIM // P  # dim chunks (16)
    assert NCH * B == 128

    # Bacc (not plain Bass): its compile() pass legalizes multi-sem waits on
    # control instructions (the Tile kernel-tail Drain), which this walrus
    # build cannot encode directly.
    nc = bacc.Bacc("TRN2")
    x_d = nc.declare_dram_parameter("input_x", [B, seq, DIM], f32, isOutput=False)
    idx_d = nc.declare_dram_parameter("lora_idx", [B, R], i32, isOutput=False)
    wb_d = nc.declare_dram_parameter("w_b", [E, DIM], f32, isOutput=False)
    emb_d = nc.declare_dram_parameter("embed", [E, DIM], f32, isOutput=False)
    wc_d = nc.declare_dram_parameter("w_c", [E, DIM], f32, isOutput=False)
    score_d = nc.declare_dram_parameter("norm_score", [B, E], f32, isOutput=True)
    topk_d = nc.declare_dram_parameter("topk", [B, R], i32, isOutput=True)

    x_r = x_d[:, :, :].rearrange("b (g t p) d -> b g p t d", t=T, p=P)

    with tile.TileContext(nc) as tc:
        with (
            tc.tile_pool(name="xin", bufs=x_bufs) as xin,
            tc.tile_pool(name="misc", bufs=1) as misc,
            tc.tile_pool(name="ps_big", bufs=1, space="PSUM") as ps_big,
            tc.tile_pool(name="ps_tr", bufs=2, space="PSUM") as ps_tr,
            tc.tile_pool(name="ps_small", bufs=2, space="PSUM") as ps_small,
        ):
            # ---- small input loads (issued first so they lead the DMA queues)
            idx_sb = misc.tile([B, R], i32)
            nc.sync.dma_start(idx_sb[:], idx_d[:, :])
            wb_sb = misc.tile([E, DIM], f32)
            nc.sync.dma_start(wb_sb[:], wb_d[:, :])
            wc_sb = misc.tile([E, DIM], f32)
            nc.sync.dma_start(wc_sb[:], wc_d[:, :])
            emb_sb = misc.tile([E, DIM], f32)
            nc.sync.dma_start(emb_sb[:], emb_d[:, :])

            # ---- constants
            ones = misc.tile([P, 1], f32)
            nc.gpsimd.memset(ones[:], 1.0)

            iota16_i = misc.tile([E, E], i32)
            nc.gpsimd.iota(iota16_i[:], [[1, E]], channel_multiplier=0)
            iota16_f = misc.tile([E, E], f32)
            nc.vector.tensor_copy(iota16_f[:], iota16_i[:])

            iotap_i = misc.tile([E, 1], i32)
            nc.gpsimd.iota(iotap_i[:], [[1, 1]], channel_multiplier=1)
            iotap_f = misc.tile([E, 1], f32)
            nc.vector.tensor_copy(iotap_f[:], iotap_i[:])

            # identity16 (f32): 1.0 where col == row
            id16 = misc.tile([E, E], f32)
            nc.vector.tensor_scalar(
                id16[:], iota16_f[:], iotap_f[:], None, op0=OP.is_equal
            )

            # ---- weight transposes: [16, 2048] -> chunks [128, 16] x 16
            def transpose_weight(w_sb, name):
                ps = ps_tr.tile([P, NCH * E], f32, tag="tr")
                for c in range(NCH):
                    nc.tensor.transpose(
                        ps[:, c * E : (c + 1) * E],
                        w_sb[:, c * P : (c + 1) * P],
                        id16[:],
                    )
                out = misc.tile([P, NCH * E], f32, tag=name)
                nc.vector.tensor_copy(out[:], ps[:])
                return out

            wbT = transpose_weight(wb_sb, "wbT")
            wcT = transpose_weight(wc_sb, "wcT")
            embT = transpose_weight(emb_sb, "embT")

            # ---- G = embed @ W_c.T  [16, 16], then scale by 1/R
            g_ps = ps_small.tile([E, E], f32, tag="small")
            for c in range(NCH):
                nc.tensor.matmul(
                    g_ps[:],
                    embT[:, c * E : (c + 1) * E],
                    wcT[:, c * E : (c + 1) * E],
                    start=(c == 0),
                    stop=(c == NCH - 1),
                )
            g_sb = misc.tile([E, E], f32)
            nc.scalar.activation(g_sb[:], g_ps[:], AF.Copy, scale=1.0 / R)

            # ---- expert counts from indices: counts[b,e] = sum_j (idx[b,j]==e)
            idx_f = misc.tile([B, R], f32)
            nc.vector.tensor_copy(idx_f[:], idx_sb[:])
            counts = misc.tile([B, E], f32)
            nc.vector.tensor_scalar(
                counts[:], iota16_f[:B, :], idx_f[:, 0:1], None, op0=OP.is_equal
            )
            cnt_tmp = misc.tile([B, E], f32)
            for j in range(1, R):
                nc.vector.tensor_scalar(
                    cnt_tmp[:], iota16_f[:B, :], idx_f[:, j : j + 1], None,
                    op0=OP.is_equal,
                )
                nc.vector.tensor_add(counts[:], counts[:], cnt_tmp[:])

            ct_ps = ps_small.tile([E, B], f32, tag="small")
            nc.tensor.transpose(ct_ps[:], counts[:], id16[:B, :B])
            ct_sb = misc.tile([E, B], f32)
            nc.vector.tensor_copy(ct_sb[:], ct_ps[:])

            # ---- stage 1: batch column sums of x into PSUM [128, NCH*B]
            xsum_ps = ps_big.tile([P, NCH * B], f32)
            for b in range(B):
                for g in range(NG):
                    xt = xin.tile([P, T * DIM], f32, tag="xt")
                    nc.sync.dma_start(
                        xt[:].rearrange("p (t d) -> p t d", t=T), x_r[b, g]
                    )
                    # PSUM has_written is per element, but start=True clears the
                    # bits for the whole bank: issue start on the first matmul
                    # only; every column's first write then lands as overwrite
                    # (bit unset) and later writes accumulate.
                    for tl in range(T):
                        t = g * T + tl
                        for c in range(NCH):
                            nc.tensor.matmul(
                                xsum_ps[:, c * B + b : c * B + b + 1],
                                xt[:, tl * DIM + c * P : tl * DIM + (c + 1) * P],
                                ones[:],
                                start=(b == 0 and t == 0 and c == 0),
                                stop=(b == B - 1 and t == NT - 1 and c == NCH - 1),
                            )

            # xmT[p, c*B+b] = mean_s x[b, s, c*128+p]
            xmT = misc.tile([P, NCH * B], f32)
            nc.scalar.activation(xmT[:], xsum_ps[:], AF.Copy, scale=1.0 / seq)

            # ---- l1 = xmean @ W_b.T   [B, E]
            l1_ps = ps_small.tile([B, E], f32, tag="small")
            for c in range(NCH):
                nc.tensor.matmul(
                    l1_ps[:],
                    xmT[:, c * B : (c + 1) * B],
                    wbT[:, c * E : (c + 1) * E],
                    start=(c == 0),
                    stop=(c == NCH - 1),
                )

            # ---- l2 = counts/R @ G    [B, E]
            l2_ps = ps_small.tile([B, E], f32, tag="small")
            nc.tensor.matmul(l2_ps[:], ct_sb[:], g_sb[:])

            # ---- logits = softmax(l1) + softmax(l2)
            def softmax_from_psum(ps_in, name):
                ex = misc.tile([B, E], f32, tag=name + "_ex")
                sm = misc.tile([B, 1], f32, tag=name + "_sum")
                nc.scalar.activation(ex[:], ps_in[:], AF.Exp, accum_out=sm[:])
                rc = misc.tile([B, 1], f32, tag=name + "_rcp")
                nc.vector.reciprocal(rc[:], sm[:])
                out = misc.tile([B, E], f32, tag=name + "_soft")
                nc.vector.tensor_scalar(out[:], ex[:], rc[:], None, op0=OP.mult)
                return out

            soft1 = softmax_from_psum(l1_ps, "s1")
            soft2 = softmax_from_psum(l2_ps, "s2")
            logits = misc.tile([B, E], f32)
            nc.vector.tensor_add(logits[:], soft1[:], soft2[:])

            # ---- top-4 + renormalized scores
            max8 = misc.tile([B, 8], f32)
            nc.vector.max(max8[:], logits[:])
            idx8 = misc.tile([B, 8], u32)
            nc.vector.max_index(idx8[:], max8[:], logits[:])

            e4 = misc.tile([B, R], f32)
            es = misc.tile([B, 1], f32)
            nc.scalar.activation(e4[:], max8[:, 0:R], AF.Exp, accum_out=es[:])
            r4 = misc.tile([B, 1], f32)
            nc.vector.reciprocal(r4[:], es[:])
            p4 = misc.tile([B, R], f32)
            nc.vector.tensor_scalar(p4[:], e4[:], r4[:], None, op0=OP.mult)

            idx8_f = misc.tile([B, R], f32)
            nc.vector.tensor_copy(idx8_f[:], idx8[:, 0:R])

            # scatter: ns[b, e] = sum_j (e == idx8[b,j]) * p4[b,j]
            oh = [misc.tile([B, E], f32, tag=f"oh{j}", name=f"oh{j}") for j in range(R)]
            for j in range(R):
                nc.vector.tensor_scalar(
                    oh[j][:], iota16_f[:B, :], idx8_f[:, j : j + 1],
                    p4[:, j : j + 1], op0=OP.is_equal, op1=OP.mult,
                )
            nc.vector.tensor_add(oh[0][:], oh[0][:], oh[1][:])
            nc.vector.tensor_add(oh[2][:], oh[2][:], oh[3][:])
            ns = misc.tile([B, E], f32)
            nc.vector.tensor_add(ns[:], oh[0][:], oh[2][:])

            topk_i = misc.tile([B, R], i32)
            nc.vector.tensor_copy(topk_i[:], idx8[:, 0:R])

            nc.sync.dma_start(score_d[:, :], ns[:])
            nc.sync.dma_start(topk_d[:, :], topk_i[:])

    nc.compile()
    return nc


_NC = None
LAST_EXEC_NS = None


def kernel(**inputs):
    global _NC, LAST_EXEC_NS
    from concourse.bass_utils import run_bass_kernel_spmd

    x = np.ascontiguousarray(np.asarray(inputs["input_x"], dtype=np.float32))
    idx = np.ascontiguousarray(np.asarray(inputs["loraA_indices"]).astype(np.int32))
    w_b = np.ascontiguousarray(np.asarray(inputs["W_b"], dtype=np.float32))
    embed = np.ascontiguousarray(np.asarray(inputs["embed"], dtype=np.float32))
    w_c = np.ascontiguousarray(np.asarray(inputs["W_c"], dtype=np.float32))

    if _NC is None:
        _NC = build_nc()

    in_maps = [
        {
            "input_x": x[i * B : (i + 1) * B],
            "lora_idx": idx[i * B : (i + 1) * B],
            "w_b": w_b,
            "embed": embed,
            "w_c": w_c,
        }
        for i in range(N_CORES)
    ]
    res = run_bass_kernel_spmd(_NC, in_maps, list(range(N_CORES)))
    LAST_EXEC_NS = res.exec_time_ns
    norm_score = np.concatenate([r["norm_score"] for r in res.results], axis=0)
    topk = np.concatenate([r["topk"] for r in res.results], axis=0)
    return norm_score, topk


# revision 9
# speedup vs baseline: 1.3842x; 1.3842x over previous
"""Trainium2 Bass kernel for the CMoALoRA2B selector (MoE routing).

Math (per the reference):
    x      = input_x.mean(axis=1)                    # [bz, dim]   <- 1 GiB read, the memory-bound part
    l1     = x @ W_b.T                               # [bz, E]
    hidden = embed[loraA_indices].mean(axis=1)       # [bz, dim]
    l2     = hidden @ W_c.T                          # [bz, E]
    logits = softmax(l1) + softmax(l2)
    topk   = top_k(logits, 4)
    out    = softmax(logits) masked to topk, renormalized
           = softmax over the top-4 logits, scattered back     (identical math)

Distribution: data-parallel over batch; 8 cores x 8 batches each. Router
weights (16 x 2048 each) are replicated. Everything runs on-device; the host
only shards inputs and concatenates the per-core [8,16]/[8,4] outputs.

Device strategy per core:
  Stage 1: stream the [8, 2048, 2048] shard in [128, 2048] tiles; each
    128x128 chunk becomes the matmul *stationary* operand against a ones
    [128,1] moving vector, accumulating column sums into a single PSUM tile
    [128, 128] laid out as col = chunk*8 + batch. This yields the transposed
    batch-means directly (dim on partitions), which the router matmuls need.
  Stage 2 (tiny): PE-transpose the 16x2048 weights into [128,16] chunks,
    G = embed @ W_c.T via PE, expert counts from indices via iota+is_equal,
    l2 = counts/4 @ G, softmaxes on ACT (exp + accum_out), top-k via the DVE
    max/max_index instructions, and scatter of the renormalized top-4 scores.
"""

import sys

for _p in ("/opt/trn_rl_repo",):
    if _p not in sys.path:
        sys.path.insert(0, _p)

import numpy as np

BZ, SEQ, DIM = 64, 2048, 2048
E, R = 16, 4
N_CORES = 8
B = BZ // N_CORES  # batches per core
P = 128  # partitions

# Tunables
SEQ_TILES_PER_DMA = 2  # T: seq tiles (128 rows) per dma_start
X_BUFS = 6  # in-flight input tiles


def build_nc(seq=SEQ, t_per_dma=SEQ_TILES_PER_DMA, x_bufs=X_BUFS, extra_read=False):
    import concourse.bass as bass
    import concourse.tile as tile
    from concourse import bacc, mybir

    f32 = mybir.dt.float32
    i32 = mybir.dt.int32
    u32 = mybir.dt.uint32
    AF = mybir.ActivationFunctionType
    OP = mybir.AluOpType

    NT = seq // P  # seq tiles per batch
    T = t_per_dma
    NG = NT // T  # dma groups per batch
    NCH = DIM // P  # dim chunks (16)
    assert NCH * B == 128

    # Bacc (not plain Bass): its compile() pass legalizes multi-sem waits on
    # control instructions (the Tile kernel-tail Drain), which this walrus
    # build cannot encode directly.
    nc = bacc.Bacc("TRN2")
    x_d = nc.declare_dram_parameter("input_x", [B, seq, DIM], f32, isOutput=False)
    idx_d = nc.declare_dram_parameter("lora_idx", [B, R], i32, isOutput=False)
    wb_d = nc.declare_dram_parameter("w_b", [E, DIM], f32, isOutput=False)
    emb_d = nc.declare_dram_parameter("embed", [E, DIM], f32, isOutput=False)
    wc_d = nc.declare_dram_parameter("w_c", [E, DIM], f32, isOutput=False)
    score_d = nc.declare_dram_parameter("norm_score", [B, E], f32, isOutput=True)
    topk_d = nc.declare_dram_parameter("topk", [B, R], i32, isOutput=True)

    x_r = x_d[:, :, :].rearrange("b (g t p) d -> b g p t d", t=T, p=P)

    with tile.TileContext(nc) as tc:
        with (
            tc.tile_pool(name="xin", bufs=x_bufs) as xin,
            tc.tile_pool(name="misc", bufs=1) as misc,
            tc.tile_pool(name="ps_big", bufs=1, space="PSUM") as ps_big,
            tc.tile_pool(name="ps_tr", bufs=2, space="PSUM") as ps_tr,
            tc.tile_pool(name="ps_small", bufs=2, space="PSUM") as ps_small,
        ):
            # ---- small input loads (issued first so they lead the DMA queues)
            idx_sb = misc.tile([B, R], i32)
            nc.sync.dma_start(idx_sb[:], idx_d[:, :])
            wb_sb = misc.tile([E, DIM], f32)
            nc.sync.dma_start(wb_sb[:], wb_d[:, :])
            wc_sb = misc.tile([E, DIM], f32)
            nc.sync.dma_start(wc_sb[:], wc_d[:, :])
            emb_sb = misc.tile([E, DIM], f32)
            nc.sync.dma_start(emb_sb[:], emb_d[:, :])

            # ---- constants
            ones = misc.tile([P, 1], f32)
            nc.gpsimd.memset(ones[:], 1.0)

            iota16_i = misc.tile([E, E], i32)
            nc.gpsimd.iota(iota16_i[:], [[1, E]], channel_multiplier=0)
            iota16_f = misc.tile([E, E], f32)
            nc.vector.tensor_copy(iota16_f[:], iota16_i[:])

            iotap_i = misc.tile([E, 1], i32)
            nc.gpsimd.iota(iotap_i[:], [[1, 1]], channel_multiplier=1)
            iotap_f = misc.tile([E, 1], f32)
            nc.vector.tensor_copy(iotap_f[:], iotap_i[:])

            # identity16 (f32): 1.0 where col == row
            id16 = misc.tile([E, E], f32)
            nc.vector.tensor_scalar(
                id16[:], iota16_f[:], iotap_f[:], None, op0=OP.is_equal
            )

            # ---- weight transposes: [16, 2048] -> chunks [128, 16] x 16
            def transpose_weight(w_sb, name):
                ps = ps_tr.tile([P, NCH * E], f32, tag="tr")
                for c in range(NCH):
                    nc.tensor.transpose(
                        ps[:, c * E : (c + 1) * E],
                        w_sb[:, c * P : (c + 1) * P],
                        id16[:],
                    )
                out = misc.tile([P, NCH * E], f32, tag=name)
                nc.vector.tensor_copy(out[:], ps[:])
                return out

            wbT = transpose_weight(wb_sb, "wbT")
            wcT = transpose_weight(wc_sb, "wcT")
            embT = transpose_weight(emb_sb, "embT")

            # ---- G = embed @ W_c.T  [16, 16], then scale by 1/R
            g_ps = ps_small.tile([E, E], f32, tag="small")
            for c in range(NCH):
                nc.tensor.matmul(
                    g_ps[:],
                    embT[:, c * E : (c + 1) * E],
                    wcT[:, c * E : (c + 1) * E],
                    start=(c == 0),
                    stop=(c == NCH - 1),
                )
            g_sb = misc.tile([E, E], f32)
            nc.scalar.activation(g_sb[:], g_ps[:], AF.Copy, scale=1.0 / R)

            # ---- expert counts from indices: counts[b,e] = sum_j (idx[b,j]==e)
            idx_f = misc.tile([B, R], f32)
            nc.vector.tensor_copy(idx_f[:], idx_sb[:])
            counts = misc.tile([B, E], f32)
            nc.vector.tensor_scalar(
                counts[:], iota16_f[:B, :], idx_f[:, 0:1], None, op0=OP.is_equal
            )
            cnt_tmp = misc.tile([B, E], f32)
            for j in range(1, R):
                nc.vector.tensor_scalar(
                    cnt_tmp[:], iota16_f[:B, :], idx_f[:, j : j + 1], None,
                    op0=OP.is_equal,
                )
                nc.vector.tensor_add(counts[:], counts[:], cnt_tmp[:])

            ct_ps = ps_small.tile([E, B], f32, tag="small")
            nc.tensor.transpose(ct_ps[:], counts[:], id16[:B, :B])
            ct_sb = misc.tile([E, B], f32)
            nc.vector.tensor_copy(ct_sb[:], ct_ps[:])

            # ---- stage 1: batch column sums of x into PSUM [128, NCH*B]
            xsum_ps = ps_big.tile([P, NCH * B], f32)
            for b in range(B):
                for g in range(NG):
                    xt = xin.tile([P, T * DIM], f32, tag="xt")
                    nc.sync.dma_start(
                        xt[:].rearrange("p (t d) -> p t d", t=T), x_r[b, g]
                    )
                    if extra_read:  # bench-only: double the HBM read traffic
                        dummy = xin.tile([P, T * DIM], f32, tag="dummy", name="dummy")
                        nc.sync.dma_start(
                            dummy[:].rearrange("p (t d) -> p t d", t=T), x_r[b, g]
                        )
                    # PSUM has_written is per element, but start=True clears the
                    # bits for the whole bank: issue start on the first matmul
                    # only; every column's first write then lands as overwrite
                    # (bit unset) and later writes accumulate.
                    for tl in range(T):
                        t = g * T + tl
                        for c in range(NCH):
                            nc.tensor.matmul(
                                xsum_ps[:, c * B + b : c * B + b + 1],
                                xt[:, tl * DIM + c * P : tl * DIM + (c + 1) * P],
                                ones[:],
                                start=(b == 0 and t == 0 and c == 0),
                                stop=(b == B - 1 and t == NT - 1 and c == NCH - 1),
                            )

            # xmT[p, c*B+b] = mean_s x[b, s, c*128+p]
            xmT = misc.tile([P, NCH * B], f32)
            nc.scalar.activation(xmT[:], xsum_ps[:], AF.Copy, scale=1.0 / seq)

            # ---- l1 = xmean @ W_b.T   [B, E]
            l1_ps = ps_small.tile([B, E], f32, tag="small")
            for c in range(NCH):
                nc.tensor.matmul(
                    l1_ps[:],
                    xmT[:, c * B : (c + 1) * B],
                    wbT[:, c * E : (c + 1) * E],
                    start=(c == 0),
                    stop=(c == NCH - 1),
                )

            # ---- l2 = counts/R @ G    [B, E]
            l2_ps = ps_small.tile([B, E], f32, tag="small")
            nc.tensor.matmul(l2_ps[:], ct_sb[:], g_sb[:])

            # ---- logits = softmax(l1) + softmax(l2)
            def softmax_from_psum(ps_in, name):
                ex = misc.tile([B, E], f32, tag=name + "_ex")
                sm = misc.tile([B, 1], f32, tag=name + "_sum")
                nc.scalar.activation(ex[:], ps_in[:], AF.Exp, accum_out=sm[:])
                rc = misc.tile([B, 1], f32, tag=name + "_rcp")
                nc.vector.reciprocal(rc[:], sm[:])
                out = misc.tile([B, E], f32, tag=name + "_soft")
                nc.vector.tensor_scalar(out[:], ex[:], rc[:], None, op0=OP.mult)
                return out

            soft1 = softmax_from_psum(l1_ps, "s1")
            soft2 = softmax_from_psum(l2_ps, "s2")
            logits = misc.tile([B, E], f32)
            nc.vector.tensor_add(logits[:], soft1[:], soft2[:])

            # ---- top-4 + renormalized scores
            max8 = misc.tile([B, 8], f32)
            nc.vector.max(max8[:], logits[:])
            idx8 = misc.tile([B, 8], u32)
            nc.vector.max_index(idx8[:], max8[:], logits[:])

            e4 = misc.tile([B, R], f32)
            es = misc.tile([B, 1], f32)
            nc.scalar.activation(e4[:], max8[:, 0:R], AF.Exp, accum_out=es[:])
            r4 = misc.tile([B, 1], f32)
            nc.vector.reciprocal(r4[:], es[:])
            p4 = misc.tile([B, R], f32)
            nc.vector.tensor_scalar(p4[:], e4[:], r4[:], None, op0=OP.mult)

            idx8_f = misc.tile([B, R], f32)
            nc.vector.tensor_copy(idx8_f[:], idx8[:, 0:R])

            # scatter: ns[b, e] = sum_j (e == idx8[b,j]) * p4[b,j]
            oh = [misc.tile([B, E], f32, tag=f"oh{j}", name=f"oh{j}") for j in range(R)]
            for j in range(R):
                nc.vector.tensor_scalar(
                    oh[j][:], iota16_f[:B, :], idx8_f[:, j : j + 1],
                    p4[:, j : j + 1], op0=OP.is_equal, op1=OP.mult,
                )
            nc.vector.tensor_add(oh[0][:], oh[0][:], oh[1][:])
            nc.vector.tensor_add(oh[2][:], oh[2][:], oh[3][:])
            ns = misc.tile([B, E], f32)
            nc.vector.tensor_add(ns[:], oh[0][:], oh[2][:])

            topk_i = misc.tile([B, R], i32)
            nc.vector.tensor_copy(topk_i[:], idx8[:, 0:R])

            nc.sync.dma_start(score_d[:, :], ns[:])
            nc.sync.dma_start(topk_d[:, :], topk_i[:])

    nc.compile()
    return nc


_NC = None
LAST_EXEC_NS = None


def kernel(**inputs):
    global _NC, LAST_EXEC_NS
    from concourse.bass_utils import run_bass_kernel_spmd

    x = np.ascontiguousarray(np.asarray(inputs["input_x"], dtype=np.float32))
    idx = np.ascontiguousarray(np.asarray(inputs["loraA_indices"]).astype(np.int32))
    w_b = np.ascontiguousarray(np.asarray(inputs["W_b"], dtype=np.float32))
    embed = np.ascontiguousarray(np.asarray(inputs["embed"], dtype=np.float32))
    w_c = np.ascontiguousarray(np.asarray(inputs["W_c"], dtype=np.float32))

    if _NC is None:
        _NC = build_nc()

    in_maps = [
        {
            "input_x": x[i * B : (i + 1) * B],
            "lora_idx": idx[i * B : (i + 1) * B],
            "w_b": w_b,
            "embed": embed,
            "w_c": w_c,
        }
        for i in range(N_CORES)
    ]
    res = run_bass_kernel_spmd(_NC, in_maps, list(range(N_CORES)))
    LAST_EXEC_NS = res.exec_time_ns
    norm_score = np.concatenate([r["norm_score"] for r in res.results], axis=0)
    topk = np.concatenate([r["topk"] for r in res.results], axis=0)
    return norm_score, topk
